# revision 3
# baseline (speedup 1.0000x reference)
"""Trainium2 Bass kernel for the YOLO-style DetectionLoss.

Full inputs in, full (scalar) output out. Internally:
  - Only the conf channels (a*8+4, i.e. 3 of 24 channels) need a full-tensor
    pass: loss_conf = mean((sigmoid(conf) - m)^2). Decompose as
       sum_all sigmoid(conf)^2  +  sum_masked [(sig-1)^2 - sig^2]
    so the bulk device work is an 8-way batch-sharded sigmoid-square-reduce
    over pred[:, 4::8] (1.23 MB/core instead of 9.8 MB/core for full pred).
  - The masked box/cls/conf-correction terms only touch the <=512 target
    cells; those 24 values/cell are gathered host-side (pure indexing) and
    evaluated on-device in one small (24, ncells) block per core:
       F = sigmoid(Vs) + exp(Ve)   (each matrix holds -100 in the columns
                                    belonging to the other nonlinearity)
       r1 = sum_cells (F - T)^2    per channel row
       r2 = sum_cells F^2          per channel row
  - Host combines the 8 cores' partial sums and applies the final divisions.
"""

import numpy as np

A = 3
NUM_CLS = 3
B, C, H, W = 32, 24, 160, 160
HW = H * W
M = 8            # cores
BPC = B // M     # batches per core
P = 128
CONF_ELEMS = BPC * A * HW        # 307200 per core
FREE = CONF_ELEMS // P           # 2400
NEG = -100.0                     # sigmoid(-100) == exp(-100) == 0 in f32

NCHUNKS = 4

TRACE = False        # test harness can flip this to get a profile
LAST = None          # BassKernelResults of the most recent run

_PROGRAM_CACHE = {}


def _build_program(ncells_pad, nchunks):
    import concourse.tile as tile
    from concourse import bacc, mybir

    f32 = mybir.dt.float32
    Act = mybir.ActivationFunctionType
    Alu = mybir.AluOpType

    nc = bacc.Bacc("TRN2", target_bir_lowering=False, debug=False, num_devices=M)

    conf_t = nc.dram_tensor("conf", [P, FREE], f32, kind="ExternalInput")
    tvs_t = nc.dram_tensor("tvs", [24, ncells_pad], f32, kind="ExternalInput")
    tve_t = nc.dram_tensor("tve", [24, ncells_pad], f32, kind="ExternalInput")
    tt_t = nc.dram_tensor("tt", [24, ncells_pad], f32, kind="ExternalInput")
    oacc_t = nc.dram_tensor("oacc", [P, nchunks], f32, kind="ExternalOutput")
    or1_t = nc.dram_tensor("or1", [24, 1], f32, kind="ExternalOutput")
    or2_t = nc.dram_tensor("or2", [24, 1], f32, kind="ExternalOutput")

    cw = FREE // nchunks
    assert cw * nchunks == FREE

    with tile.TileContext(nc) as tc:
        with (
            tc.tile_pool(name="x", bufs=3) as xp,
            tc.tile_pool(name="s", bufs=3) as sp,
            tc.tile_pool(name="scr", bufs=2) as scrp,
            tc.tile_pool(name="acc", bufs=1) as accp,
            tc.tile_pool(name="tgt", bufs=1) as tp,
        ):
            # ---- bulk: sum of sigmoid(conf)^2, chunked over the free dim ----
            acc = accp.tile([P, nchunks], f32)
            for i in range(nchunks):
                x = xp.tile([P, cw], f32)
                nc.sync.dma_start(x[:], conf_t.ap()[:, i * cw:(i + 1) * cw])
                s = sp.tile([P, cw], f32)
                nc.scalar.activation(s[:], x[:], Act.Sigmoid)
                sq = scrp.tile([P, cw], f32)
                nc.vector.scalar_tensor_tensor(
                    out=sq[:], in0=s[:], scalar=0.0, in1=s[:],
                    op0=Alu.add, op1=Alu.mult, accum_out=acc[:, i:i + 1],
                )
            nc.sync.dma_start(oacc_t.ap()[:], acc[:])

            # ---- masked cells: (24, ncells_pad) block ----
            vs = tp.tile([24, ncells_pad], f32)
            nc.sync.dma_start(vs[:], tvs_t.ap()[:])
            ve = tp.tile([24, ncells_pad], f32)
            nc.sync.dma_start(ve[:], tve_t.ap()[:])
            tt = tp.tile([24, ncells_pad], f32)
            nc.sync.dma_start(tt[:], tt_t.ap()[:])

            sm = tp.tile([24, ncells_pad], f32)
            nc.scalar.activation(sm[:], vs[:], Act.Sigmoid)
            em = tp.tile([24, ncells_pad], f32)
            nc.scalar.activation(em[:], ve[:], Act.Exp)
            fm = tp.tile([24, ncells_pad], f32)
            nc.vector.tensor_add(fm[:], sm[:], em[:])
            dm = tp.tile([24, ncells_pad], f32)
            nc.vector.tensor_sub(dm[:], fm[:], tt[:])

            r1 = tp.tile([24, 1], f32)
            scr1 = tp.tile([24, ncells_pad], f32)
            nc.scalar.activation(scr1[:], dm[:], Act.Square, accum_out=r1[:])
            r2 = tp.tile([24, 1], f32)
            scr2 = tp.tile([24, ncells_pad], f32)
            nc.scalar.activation(scr2[:], fm[:], Act.Square, accum_out=r2[:])
            nc.sync.dma_start(or1_t.ap()[:], r1[:])
            nc.sync.dma_start(or2_t.ap()[:], r2[:])

    nc.compile()
    return nc


def _get_program(ncells_pad, nchunks):
    key = (ncells_pad, nchunks)
    if key not in _PROGRAM_CACHE:
        _PROGRAM_CACHE[key] = _build_program(ncells_pad, nchunks)
    return _PROGRAM_CACHE[key]


def kernel(pred, targets):
    global LAST
    from concourse.bass_utils import run_bass_kernel_spmd

    pred = np.ascontiguousarray(np.asarray(pred, dtype=np.float32))
    targets = np.asarray(targets, dtype=np.float32)
    assert pred.shape == (B, C, H, W), pred.shape
    N = targets.shape[0]

    # ---- host: parse targets, dedupe cells (last writer wins) ----
    b = targets[:, 0].astype(np.int32)
    c = targets[:, 1].astype(np.int32)
    gix = (targets[:, 2] * W).astype(np.int32)
    giy = (targets[:, 3] * H).astype(np.int32)
    valid = (gix < W) & (giy < H) & (gix >= 0) & (giy >= 0) & (b >= 0) & (b < B)

    cell_map = {}
    for i in range(N):
        if valid[i]:
            cell_map[(int(b[i]), int(giy[i]), int(gix[i]))] = i
    n_cells = len(cell_map)
    n = 3.0 * n_cells

    per_core = [[] for _ in range(M)]
    for (bb, yy, xx), i in cell_map.items():
        per_core[bb // BPC].append((bb, yy, xx, i))

    max_cells = max((len(pc) for pc in per_core), default=0)
    ncells_pad = max(32, ((max_cells + 31) // 32) * 32)

    # ---- host: build per-core shards ----
    pr = pred.reshape(B, A, 8, H, W)
    conf_all = pr[:, :, 4, :, :]  # (B, A, H, W)

    SIG_COL = np.array([k in (0, 1, 4, 5, 6, 7) for k in range(8)] * A)  # (24,)
    EXP_COL = ~SIG_COL

    in_maps = []
    for m in range(M):
        shard = np.ascontiguousarray(
            conf_all[m * BPC:(m + 1) * BPC]).reshape(P, FREE)

        cells = per_core[m]
        tvs = np.full((24, ncells_pad), NEG, np.float32)
        tve = np.full((24, ncells_pad), NEG, np.float32)
        tt = np.zeros((24, ncells_pad), np.float32)
        if cells:
            bbs = np.array([e[0] for e in cells])
            yys = np.array([e[1] for e in cells])
            xxs = np.array([e[2] for e in cells])
            idx = np.array([e[3] for e in cells])
            vals = pred[bbs, :, yys, xxs].T  # (24, ncells)
            ncol = len(cells)
            tvs[:, :ncol] = np.where(SIG_COL[:, None], vals, NEG)
            tve[:, :ncol] = np.where(EXP_COL[:, None], vals, NEG)
            boxes = targets[idx, 2:6].T  # (4, ncells): gx, gy, gw, gh
            onehot = np.zeros((NUM_CLS, ncol), np.float32)
            ci = c[idx]
            ok = (ci >= 0) & (ci < NUM_CLS)
            onehot[ci[ok], np.nonzero(ok)[0]] = 1.0
            for a in range(A):
                tt[a * 8 + 0:a * 8 + 4, :ncol] = boxes
                tt[a * 8 + 4, :ncol] = 1.0
                tt[a * 8 + 5:a * 8 + 8, :ncol] = onehot
        in_maps.append({"conf": shard, "tvs": tvs, "tve": tve, "tt": tt})

    # ---- device ----
    nc = _get_program(ncells_pad, NCHUNKS)
    res = run_bass_kernel_spmd(nc, in_maps, list(range(M)), trace=TRACE)
    LAST = res

    # ---- host: combine ----
    S2 = 0.0
    r1_tot = np.zeros(24, np.float64)
    r2_tot = np.zeros(24, np.float64)
    for m in range(M):
        out = res.results[m]
        S2 += float(out["oacc"].astype(np.float64).sum())
        r1_tot += out["or1"][:, 0].astype(np.float64)
        r2_tot += out["or2"][:, 0].astype(np.float64)

    box_rows = [a * 8 + k for a in range(A) for k in range(4)]
    conf_rows = [a * 8 + 4 for a in range(A)]
    cls_rows = [a * 8 + k for a in range(A) for k in range(5, 8)]

    box_sum = r1_tot[box_rows].sum()
    cls_sum = r1_tot[cls_rows].sum()
    conf_corr = (r1_tot[conf_rows] - r2_tot[conf_rows]).sum()

    with np.errstate(divide="ignore", invalid="ignore"):
        loss_box = box_sum / (n * 4.0)
        loss_conf = (S2 + conf_corr) / float(B * A * HW)
        loss_cls = cls_sum / (n * NUM_CLS)
        total = 5.0 * loss_box + loss_conf + loss_cls
    return np.asarray(total, dtype=np.float32)


# revision 13
# speedup vs baseline: 1.1585x; 1.1585x over previous
"""Trainium2 Bass kernel for the YOLO-style DetectionLoss.

Full inputs in, full (scalar) output out. Internally:
  - Only the conf channels (a*8+4, i.e. 3 of 24 channels) need a full-tensor
    pass: loss_conf = mean((sigmoid(conf) - m)^2). Decompose as
       sum_all sigmoid(conf)^2  +  sum_masked [(sig-1)^2 - sig^2]
    so the bulk device work is an 8-way batch-sharded sigmoid-square-reduce
    over pred[:, 4::8] (1.23 MB/core instead of 9.8 MB/core for full pred).
  - The masked box/cls/conf-correction terms only touch the <=512 target
    cells; those 24 values/cell are gathered host-side (pure indexing) and
    evaluated on-device in one small (72, ncells) block per core. To keep
    the ACT engine on a single function table (table switches cost ~1.3us),
    exp(v) is computed as 1/sigmoid(-v) - 1 via the DVE reciprocal:
       rows  0..23: u = v on sigmoid-cols else -100   -> S = sig(u)
       rows 24..47: q = -v on exp-cols   else +100    -> R = 1/sig(q)
       rows 48..71: T = targets (pad cols 0)
       F = (R - 1) + S        (exp cols: e^v, sigmoid cols: sig(v))
       r1[row] = sum_cells (F - T)^2 ;  r2[row] = sum_cells F^2
  - Host combines the 8 cores' partial sums and applies the final divisions.
"""

import numpy as np

A = 3
NUM_CLS = 3
B, C, H, W = 32, 24, 160, 160
HW = H * W
M = 8            # cores
BPC = B // M     # batches per core
P = 128
CONF_ELEMS = BPC * A * HW        # 307200 per core
FREE = CONF_ELEMS // P           # 2400
NEG = -100.0                     # sigmoid(-100) == 0, sigmoid(+100) == 1 in f32

NCHUNKS = 4
ALT_RINGS = True   # alternate bulk-chunk DMA descriptor-gen between SP and ACT rings

TRACE = False        # test harness can flip this to get a profile
LAST = None          # BassKernelResults of the most recent run

_PROGRAM_CACHE = {}


def _build_program(ncells_pad, nchunks):
    import concourse.tile as tile
    from concourse import bacc, mybir

    f32 = mybir.dt.float32
    Act = mybir.ActivationFunctionType
    Alu = mybir.AluOpType

    nc = bacc.Bacc("TRN2", target_bir_lowering=False, debug=False, num_devices=M)

    conf_t = nc.dram_tensor("conf", [P, FREE], f32, kind="ExternalInput")
    # rows 0-23: u, rows 24-47: q, rows 48-71: T
    tin_t = nc.dram_tensor("tin", [72, ncells_pad], f32, kind="ExternalInput")
    oall_t = nc.dram_tensor("oall", [P, nchunks + 2], f32, kind="ExternalOutput")

    cw = FREE // nchunks
    assert cw * nchunks == FREE

    with tile.TileContext(nc) as tc:
        with (
            tc.tile_pool(name="x", bufs=3) as xp,
            tc.tile_pool(name="s", bufs=3) as sp,
            tc.tile_pool(name="scr", bufs=2) as scrp,
            tc.tile_pool(name="acc", bufs=1) as accp,
            tc.tile_pool(name="tgt", bufs=1) as tp,
        ):
            acc = accp.tile([P, nchunks + 2], f32)

            # ---- masked cells: (24, ncells_pad) tiles, all at base partition 0 ----
            tu = tp.tile([24, ncells_pad], f32)
            nc.gpsimd.dma_start(tu[:], tin_t.ap()[0:24, :])
            tq = tp.tile([24, ncells_pad], f32)
            nc.gpsimd.dma_start(tq[:], tin_t.ap()[24:48, :])
            ttg = tp.tile([24, ncells_pad], f32)
            nc.gpsimd.dma_start(ttg[:], tin_t.ap()[48:72, :])
            s1 = tp.tile([24, ncells_pad], f32)
            nc.scalar.activation(s1[:], tu[:], Act.Sigmoid)
            s2 = tp.tile([24, ncells_pad], f32)
            nc.scalar.activation(s2[:], tq[:], Act.Sigmoid)
            rc = tp.tile([24, ncells_pad], f32)
            nc.vector.reciprocal(rc[:], s2[:])
            fm = tp.tile([24, ncells_pad], f32)
            nc.vector.scalar_tensor_tensor(
                out=fm[:], in0=rc[:], scalar=-1.0, in1=s1[:],
                op0=Alu.add, op1=Alu.add)
            dm = tp.tile([24, ncells_pad], f32)
            nc.vector.scalar_tensor_tensor(
                out=dm[:], in0=fm[:], scalar=0.0, in1=ttg[:],
                op0=Alu.add, op1=Alu.subtract)
            t1 = tp.tile([24, ncells_pad], f32)
            nc.vector.scalar_tensor_tensor(
                out=t1[:], in0=dm[:], scalar=0.0, in1=dm[:],
                op0=Alu.add, op1=Alu.mult, accum_out=acc[0:24, nchunks:nchunks + 1])
            t2 = tp.tile([24, ncells_pad], f32)
            nc.vector.scalar_tensor_tensor(
                out=t2[:], in0=fm[:], scalar=0.0, in1=fm[:],
                op0=Alu.add, op1=Alu.mult,
                accum_out=acc[0:24, nchunks + 1:nchunks + 2])

            # ---- bulk: sum of sigmoid(conf)^2, chunked over the free dim ----
            for i in range(nchunks):
                x = xp.tile([P, cw], f32)
                eng = nc.scalar if (ALT_RINGS and i % 2) else nc.sync
                eng.dma_start(x[:], conf_t.ap()[:, i * cw:(i + 1) * cw])
                s = sp.tile([P, cw], f32)
                nc.scalar.activation(s[:], x[:], Act.Sigmoid)
                sq = scrp.tile([P, cw], f32)
                nc.vector.scalar_tensor_tensor(
                    out=sq[:], in0=s[:], scalar=0.0, in1=s[:],
                    op0=Alu.add, op1=Alu.mult, accum_out=acc[:, i:i + 1])

            nc.gpsimd.dma_start(oall_t.ap()[:], acc[:])

    nc.compile()
    return nc


def _get_program(ncells_pad, nchunks):
    key = (ncells_pad, nchunks)
    if key not in _PROGRAM_CACHE:
        _PROGRAM_CACHE[key] = _build_program(ncells_pad, nchunks)
    return _PROGRAM_CACHE[key]


def kernel(pred, targets):
    global LAST
    from concourse.bass_utils import run_bass_kernel_spmd

    pred = np.ascontiguousarray(np.asarray(pred, dtype=np.float32))
    targets = np.asarray(targets, dtype=np.float32)
    assert pred.shape == (B, C, H, W), pred.shape
    N = targets.shape[0]

    # ---- host: parse targets, dedupe cells (last writer wins) ----
    b = targets[:, 0].astype(np.int32)
    c = targets[:, 1].astype(np.int32)
    gix = (targets[:, 2] * W).astype(np.int32)
    giy = (targets[:, 3] * H).astype(np.int32)
    valid = (gix < W) & (giy < H) & (gix >= 0) & (giy >= 0) & (b >= 0) & (b < B)

    cell_map = {}
    for i in range(N):
        if valid[i]:
            cell_map[(int(b[i]), int(giy[i]), int(gix[i]))] = i
    n_cells = len(cell_map)
    n = 3.0 * n_cells

    per_core = [[] for _ in range(M)]
    for (bb, yy, xx), i in cell_map.items():
        per_core[bb // BPC].append((bb, yy, xx, i))

    max_cells = max((len(pc) for pc in per_core), default=0)
    ncells_pad = max(32, ((max_cells + 31) // 32) * 32)

    # ---- host: build per-core shards ----
    pr = pred.reshape(B, A, 8, H, W)
    conf_all = pr[:, :, 4, :, :]  # (B, A, H, W)

    SIG_COL = np.array([k in (0, 1, 4, 5, 6, 7) for k in range(8)] * A)  # (24,)

    in_maps = []
    for m in range(M):
        shard = np.ascontiguousarray(
            conf_all[m * BPC:(m + 1) * BPC]).reshape(P, FREE)

        cells = per_core[m]
        tin = np.empty((72, ncells_pad), np.float32)
        tin[0:24] = NEG     # u pad -> sig = 0
        tin[24:48] = -NEG   # q pad -> sig = 1 -> R-1 = 0
        tin[48:72] = 0.0    # T pad
        if cells:
            bbs = np.array([e[0] for e in cells])
            yys = np.array([e[1] for e in cells])
            xxs = np.array([e[2] for e in cells])
            idx = np.array([e[3] for e in cells])
            vals = pred[bbs, :, yys, xxs].T  # (24, ncells)
            ncol = len(cells)
            tin[0:24, :ncol] = np.where(SIG_COL[:, None], vals, NEG)
            tin[24:48, :ncol] = np.where(SIG_COL[:, None], -NEG, -vals)
            boxes = targets[idx, 2:6].T  # (4, ncells): gx, gy, gw, gh
            onehot = np.zeros((NUM_CLS, ncol), np.float32)
            ci = c[idx]
            ok = (ci >= 0) & (ci < NUM_CLS)
            onehot[ci[ok], np.nonzero(ok)[0]] = 1.0
            for a in range(A):
                tin[48 + a * 8 + 0:48 + a * 8 + 4, :ncol] = boxes
                tin[48 + a * 8 + 4, :ncol] = 1.0
                tin[48 + a * 8 + 5:48 + a * 8 + 8, :ncol] = onehot
        in_maps.append({"conf": shard, "tin": tin})

    # ---- device ----
    nc = _get_program(ncells_pad, NCHUNKS)
    res = run_bass_kernel_spmd(nc, in_maps, list(range(M)), trace=TRACE)
    LAST = res

    # ---- host: combine ----
    S2 = 0.0
    r1_tot = np.zeros(24, np.float64)
    r2_tot = np.zeros(24, np.float64)
    for m in range(M):
        out = res.results[m]["oall"].astype(np.float64)
        S2 += out[:, :NCHUNKS].sum()
        r1_tot += out[0:24, NCHUNKS]
        r2_tot += out[0:24, NCHUNKS + 1]

    box_rows = [a * 8 + k for a in range(A) for k in range(4)]
    conf_rows = [a * 8 + 4 for a in range(A)]
    cls_rows = [a * 8 + k for a in range(A) for k in range(5, 8)]

    box_sum = r1_tot[box_rows].sum()
    cls_sum = r1_tot[cls_rows].sum()
    conf_corr = (r1_tot[conf_rows] - r2_tot[conf_rows]).sum()

    with np.errstate(divide="ignore", invalid="ignore"):
        loss_box = box_sum / (n * 4.0)
        loss_conf = (S2 + conf_corr) / float(B * A * HW)
        loss_cls = cls_sum / (n * NUM_CLS)
        total = 5.0 * loss_box + loss_conf + loss_cls
    return np.asarray(total, dtype=np.float32)


# revision 17
# speedup vs baseline: 1.3566x; 1.1710x over previous
"""Trainium2 Bass kernel for the YOLO-style DetectionLoss.

Full inputs in, full (scalar) output out. Internally:
  - Only the conf channels (a*8+4, i.e. 3 of 24 channels) need a full-tensor
    pass: loss_conf = mean((sigmoid(conf) - m)^2). Decompose as
       sum_all sigmoid(conf)^2  +  sum_masked [(sig-1)^2 - sig^2]
    so the bulk device work is an 8-way batch-sharded sigmoid-square-reduce
    over pred[:, 4::8] (1.23 MB/core instead of 9.8 MB/core for full pred).
  - The masked box/cls/conf-correction terms only touch the <=512 target
    cells; those 24 values/cell are gathered host-side (pure indexing) and
    evaluated on-device in one small (72, ncells) block per core. To keep
    the ACT engine on a single function table (table switches cost ~1.3us),
    exp(v) is computed as 1/sigmoid(-v) - 1 via the DVE reciprocal:
       rows  0..23: u = v on sigmoid-cols else -100   -> S = sig(u)
       rows 24..47: q = -v on exp-cols   else +100    -> R = 1/sig(q)
       rows 48..71: T = targets (pad cols 0)
       F = (R - 1) + S        (exp cols: e^v, sigmoid cols: sig(v))
       r1[row] = sum_cells (F - T)^2 ;  r2[row] = sum_cells F^2
  - Host combines the 8 cores' partial sums and applies the final divisions.
"""

import numpy as np

A = 3
NUM_CLS = 3
B, C, H, W = 32, 24, 160, 160
HW = H * W
M = 8            # cores
BPC = B // M     # batches per core
P = 128
CONF_ELEMS = BPC * A * HW        # 307200 per core
FREE = CONF_ELEMS // P           # 2400
NEG = -100.0                     # sigmoid(-100) == 0, sigmoid(+100) == 1 in f32

NCHUNKS = 4
TAIL_MODE = 1      # 0 = stock Tile tail; 1 = single sem-only barrier, no 2nd barrier

TRACE = False        # test harness can flip this to get a profile
LAST = None          # BassKernelResults of the most recent run

_PROGRAM_CACHE = {}


def _make_tile_context(nc):
    """TileContext whose end-of-kernel drain is a single lightweight barrier.

    The stock tail is sync-drain + full EVSEM butterfly + sem clear + second
    butterfly (~8us of measured kernel time). All of our cross-engine sync is
    already expressed through tile semaphores, so one barrier after the
    global-clock drain is enough to make the sem clear safe; the runtime's own
    end-of-NEFF completion handles everything after that.
    """
    import concourse.tile as tile
    from concourse.vector_clock import ScopedClock

    class _FastTailTileContext(tile.TileContext):
        def _drain_and_barrier(self, tick_clock, wait_clock):
            if TAIL_MODE == 0:
                return super()._drain_and_barrier(tick_clock, wait_clock)
            drain_inst = self.nc.sync.drain()
            wait_clock.add_sem_waits(
                drain_inst.ins, ScopedClock({None: tick_clock.global_clock})
            )
            self.nc.all_engine_barrier(sem_only=True)
            popped = self.nc._tile_sem_poison_stack.pop()
            assert popped is self._sem_poison
            self.nc.clear_and_free_semaphores(
                list(self.sems.allocated().values())
            )

    return _FastTailTileContext(nc)


def _build_program(ncells_pad, nchunks):
    from concourse import bacc, mybir

    f32 = mybir.dt.float32
    Act = mybir.ActivationFunctionType
    Alu = mybir.AluOpType

    nc = bacc.Bacc("TRN2", target_bir_lowering=False, debug=False, num_devices=M)

    conf_t = nc.dram_tensor("conf", [P, FREE], f32, kind="ExternalInput")
    # rows 0-23: u, rows 24-47: q, rows 48-71: T
    tin_t = nc.dram_tensor("tin", [72, ncells_pad], f32, kind="ExternalInput")
    oall_t = nc.dram_tensor("oall", [P, nchunks + 2], f32, kind="ExternalOutput")

    cw = FREE // nchunks
    assert cw * nchunks == FREE

    with _make_tile_context(nc) as tc:
        with (
            tc.tile_pool(name="x", bufs=3) as xp,
            tc.tile_pool(name="s", bufs=3) as sp,
            tc.tile_pool(name="scr", bufs=2) as scrp,
            tc.tile_pool(name="acc", bufs=1) as accp,
            tc.tile_pool(name="tgt", bufs=1) as tp,
        ):
            acc = accp.tile([P, nchunks + 2], f32)

            # ---- masked cells: (24, ncells_pad) tiles, all at base partition 0 ----
            # small DMAs go on the ACT HWDGE ring: cheap descriptor gen at the
            # head of the scalar stream, landing before the first bulk chunk
            tu = tp.tile([24, ncells_pad], f32)
            nc.scalar.dma_start(tu[:], tin_t.ap()[0:24, :])
            tq = tp.tile([24, ncells_pad], f32)
            nc.scalar.dma_start(tq[:], tin_t.ap()[24:48, :])
            ttg = tp.tile([24, ncells_pad], f32)
            nc.scalar.dma_start(ttg[:], tin_t.ap()[48:72, :])
            s1 = tp.tile([24, ncells_pad], f32)
            nc.scalar.activation(s1[:], tu[:], Act.Sigmoid)
            s2 = tp.tile([24, ncells_pad], f32)
            nc.scalar.activation(s2[:], tq[:], Act.Sigmoid)
            rc = tp.tile([24, ncells_pad], f32)
            nc.vector.reciprocal(rc[:], s2[:])
            fm = tp.tile([24, ncells_pad], f32)
            nc.vector.scalar_tensor_tensor(
                out=fm[:], in0=rc[:], scalar=-1.0, in1=s1[:],
                op0=Alu.add, op1=Alu.add)
            dm = tp.tile([24, ncells_pad], f32)
            nc.vector.scalar_tensor_tensor(
                out=dm[:], in0=fm[:], scalar=0.0, in1=ttg[:],
                op0=Alu.add, op1=Alu.subtract)
            t1 = tp.tile([24, ncells_pad], f32)
            nc.vector.scalar_tensor_tensor(
                out=t1[:], in0=dm[:], scalar=0.0, in1=dm[:],
                op0=Alu.add, op1=Alu.mult, accum_out=acc[0:24, nchunks:nchunks + 1])
            t2 = tp.tile([24, ncells_pad], f32)
            nc.vector.scalar_tensor_tensor(
                out=t2[:], in0=fm[:], scalar=0.0, in1=fm[:],
                op0=Alu.add, op1=Alu.mult,
                accum_out=acc[0:24, nchunks + 1:nchunks + 2])

            # ---- bulk: sum of sigmoid(conf)^2, chunked over the free dim ----
            for i in range(nchunks):
                x = xp.tile([P, cw], f32)
                nc.sync.dma_start(x[:], conf_t.ap()[:, i * cw:(i + 1) * cw])
                s = sp.tile([P, cw], f32)
                nc.scalar.activation(s[:], x[:], Act.Sigmoid)
                sq = scrp.tile([P, cw], f32)
                nc.vector.scalar_tensor_tensor(
                    out=sq[:], in0=s[:], scalar=0.0, in1=s[:],
                    op0=Alu.add, op1=Alu.mult, accum_out=acc[:, i:i + 1])

            nc.sync.dma_start(oall_t.ap()[:], acc[:])

    nc.compile()
    return nc


def _get_program(ncells_pad, nchunks):
    key = (ncells_pad, nchunks)
    if key not in _PROGRAM_CACHE:
        _PROGRAM_CACHE[key] = _build_program(ncells_pad, nchunks)
    return _PROGRAM_CACHE[key]


def kernel(pred, targets):
    global LAST
    from concourse.bass_utils import run_bass_kernel_spmd

    pred = np.ascontiguousarray(np.asarray(pred, dtype=np.float32))
    targets = np.asarray(targets, dtype=np.float32)
    assert pred.shape == (B, C, H, W), pred.shape
    N = targets.shape[0]

    # ---- host: parse targets, dedupe cells (last writer wins) ----
    b = targets[:, 0].astype(np.int32)
    c = targets[:, 1].astype(np.int32)
    gix = (targets[:, 2] * W).astype(np.int32)
    giy = (targets[:, 3] * H).astype(np.int32)
    valid = (gix < W) & (giy < H) & (gix >= 0) & (giy >= 0) & (b >= 0) & (b < B)

    cell_map = {}
    for i in range(N):
        if valid[i]:
            cell_map[(int(b[i]), int(giy[i]), int(gix[i]))] = i
    n_cells = len(cell_map)
    n = 3.0 * n_cells

    per_core = [[] for _ in range(M)]
    for (bb, yy, xx), i in cell_map.items():
        per_core[bb // BPC].append((bb, yy, xx, i))

    max_cells = max((len(pc) for pc in per_core), default=0)
    ncells_pad = max(32, ((max_cells + 31) // 32) * 32)

    # ---- host: build per-core shards ----
    pr = pred.reshape(B, A, 8, H, W)
    conf_all = pr[:, :, 4, :, :]  # (B, A, H, W)

    SIG_COL = np.array([k in (0, 1, 4, 5, 6, 7) for k in range(8)] * A)  # (24,)

    in_maps = []
    for m in range(M):
        shard = np.ascontiguousarray(
            conf_all[m * BPC:(m + 1) * BPC]).reshape(P, FREE)

        cells = per_core[m]
        tin = np.empty((72, ncells_pad), np.float32)
        tin[0:24] = NEG     # u pad -> sig = 0
        tin[24:48] = -NEG   # q pad -> sig = 1 -> R-1 = 0
        tin[48:72] = 0.0    # T pad
        if cells:
            bbs = np.array([e[0] for e in cells])
            yys = np.array([e[1] for e in cells])
            xxs = np.array([e[2] for e in cells])
            idx = np.array([e[3] for e in cells])
            vals = pred[bbs, :, yys, xxs].T  # (24, ncells)
            ncol = len(cells)
            tin[0:24, :ncol] = np.where(SIG_COL[:, None], vals, NEG)
            tin[24:48, :ncol] = np.where(SIG_COL[:, None], -NEG, -vals)
            boxes = targets[idx, 2:6].T  # (4, ncells): gx, gy, gw, gh
            onehot = np.zeros((NUM_CLS, ncol), np.float32)
            ci = c[idx]
            ok = (ci >= 0) & (ci < NUM_CLS)
            onehot[ci[ok], np.nonzero(ok)[0]] = 1.0
            for a in range(A):
                tin[48 + a * 8 + 0:48 + a * 8 + 4, :ncol] = boxes
                tin[48 + a * 8 + 4, :ncol] = 1.0
                tin[48 + a * 8 + 5:48 + a * 8 + 8, :ncol] = onehot
        in_maps.append({"conf": shard, "tin": tin})

    # ---- device ----
    nc = _get_program(ncells_pad, NCHUNKS)
    res = run_bass_kernel_spmd(nc, in_maps, list(range(M)), trace=TRACE)
    LAST = res

    # ---- host: combine ----
    S2 = 0.0
    r1_tot = np.zeros(24, np.float64)
    r2_tot = np.zeros(24, np.float64)
    for m in range(M):
        out = res.results[m]["oall"].astype(np.float64)
        S2 += out[:, :NCHUNKS].sum()
        r1_tot += out[0:24, NCHUNKS]
        r2_tot += out[0:24, NCHUNKS + 1]

    box_rows = [a * 8 + k for a in range(A) for k in range(4)]
    conf_rows = [a * 8 + 4 for a in range(A)]
    cls_rows = [a * 8 + k for a in range(A) for k in range(5, 8)]

    box_sum = r1_tot[box_rows].sum()
    cls_sum = r1_tot[cls_rows].sum()
    conf_corr = (r1_tot[conf_rows] - r2_tot[conf_rows]).sum()

    with np.errstate(divide="ignore", invalid="ignore"):
        loss_box = box_sum / (n * 4.0)
        loss_conf = (S2 + conf_corr) / float(B * A * HW)
        loss_cls = cls_sum / (n * NUM_CLS)
        total = 5.0 * loss_box + loss_conf + loss_cls
    return np.asarray(total, dtype=np.float32)


# revision 18
# speedup vs baseline: 1.6680x; 1.2295x over previous
"""Trainium2 Bass kernel for the YOLO-style DetectionLoss.

Full inputs in, full (scalar) output out. Internally:
  - Only the conf channels (a*8+4, i.e. 3 of 24 channels) need a full-tensor
    pass: loss_conf = mean((sigmoid(conf) - m)^2). Decompose as
       sum_all sigmoid(conf)^2  +  sum_masked [(sig-1)^2 - sig^2]
    so the bulk device work is an 8-way batch-sharded sigmoid-square-reduce
    over pred[:, 4::8] (1.23 MB/core instead of 9.8 MB/core for full pred).
  - The masked box/cls/conf-correction terms only touch the <=512 target
    cells; those 24 values/cell are gathered host-side (pure indexing) and
    evaluated on-device in one small (24, 3*ncells) block per core laid out
    as [u | q | T] along the free axis. To keep the ACT engine on a single
    function table (table switches cost ~1.3us), exp(v) is computed as
    1/sigmoid(-v) - 1 via the DVE reciprocal:
       u = v on sigmoid-cols else -100    -> sig(u) = sig(v) or 0
       q = -v on exp-cols   else +100     -> 1/sig(q) - 1 = e^v or 0
       F = (1/sig(q) - 1) + sig(u);  per-channel-row reductions:
       r1[row] = sum_cells (F - T)^2 ;  r2[row] = sum_cells F^2
  - Host combines the 8 cores' partial sums and applies the final divisions.

Perf notes (measured on trn2 via ntff profiles):
  - The stock TileContext tail (drain + EVSEM butterfly + sem clear +
    butterfly) serializes every engine at the end, and the runtime epilogue
    then resets the full 256-sem space (~50 sems/engine, ~60-115ns each)
    AFTER that rendezvous. Skipping the Tile tail entirely lets idle
    engines run their epilogue resets during the kernel body. The runtime
    epilogue re-zeroes every semaphore, so re-execution stays correct.
  - A second act-table load (set 0) is inserted at entry by the fixpoint
    pass even though only the sigmoid table (set 2) is used; it is dropped.
"""

import numpy as np

A = 3
NUM_CLS = 3
B, C, H, W = 32, 24, 160, 160
HW = H * W
M = 8            # cores
BPC = B // M     # batches per core
P = 128
CONF_ELEMS = BPC * A * HW        # 307200 per core
FREE = CONF_ELEMS // P           # 2400
NEG = -100.0                     # sigmoid(-100) == 0, sigmoid(+100) == 1 in f32

NCHUNKS = 4
TAIL_MODE = 2      # 0 = stock Tile tail; 1 = sem-only barrier; 2 = no tail
DROP_TABLE0 = True

TRACE = False        # test harness can flip this to get a profile
LAST = None          # BassKernelResults of the most recent run

_PROGRAM_CACHE = {}


def _make_tile_context(nc):
    import concourse.tile as tile
    from concourse.vector_clock import ScopedClock

    class _FastTailTileContext(tile.TileContext):
        def _drain_and_barrier(self, tick_clock, wait_clock):
            if TAIL_MODE == 0:
                return super()._drain_and_barrier(tick_clock, wait_clock)
            if TAIL_MODE == 1:
                drain_inst = self.nc.sync.drain()
                wait_clock.add_sem_waits(
                    drain_inst.ins, ScopedClock({None: tick_clock.global_clock})
                )
                self.nc.all_engine_barrier(sem_only=True)
                popped = self.nc._tile_sem_poison_stack.pop()
                assert popped is self._sem_poison
                self.nc.clear_and_free_semaphores(
                    list(self.sems.allocated().values())
                )
                return
            # TAIL_MODE == 2: no in-kernel tail at all. In-body semaphores
            # already order every data dependency (incl. the output DMA);
            # NEFF completion itself waits for engine streams + DMA queues,
            # and the runtime epilogue zeroes the whole semaphore space.
            popped = self.nc._tile_sem_poison_stack.pop()
            assert popped is self._sem_poison

    return _FastTailTileContext(nc)


def _make_bacc():
    from concourse import bacc, mybir

    class _Bacc(bacc.Bacc):
        def insert_act_table_loads(self):
            super().insert_act_table_loads()
            if not DROP_TABLE0:
                return
            # The entry-state fixpoint conservatively loads table set 0, but
            # every activation here is Sigmoid (set 2), which gets its own
            # load. Drop the set-0 load (1.28us on the ACT engine).
            for blk in self.main_func.blocks:
                keep = []
                for inst in blk.instructions:
                    if (
                        isinstance(inst, mybir.InstLoadActFuncSet)
                        and inst.act_func_set_id == 0
                        and not (
                            inst.sync_info
                            and (inst.sync_info.on_wait or inst.sync_info.on_update)
                        )
                    ):
                        continue
                    keep.append(inst)
                blk.instructions[:] = keep

    return _Bacc("TRN2", target_bir_lowering=False, debug=False, num_devices=M)


def _build_program(ncells_pad, nchunks):
    from concourse import mybir

    f32 = mybir.dt.float32
    Act = mybir.ActivationFunctionType
    Alu = mybir.AluOpType

    nc = _make_bacc()

    NC = ncells_pad
    conf_t = nc.dram_tensor("conf", [P, FREE], f32, kind="ExternalInput")
    # columns [0:NC]=u, [NC:2NC]=q, [2NC:3NC]=T
    tin_t = nc.dram_tensor("tin", [24, 3 * NC], f32, kind="ExternalInput")
    oall_t = nc.dram_tensor("oall", [P, nchunks + 2], f32, kind="ExternalOutput")

    cw = FREE // nchunks
    assert cw * nchunks == FREE

    with _make_tile_context(nc) as tc:
        with (
            tc.tile_pool(name="x", bufs=4) as xp,
            tc.tile_pool(name="s", bufs=3) as sp,
            tc.tile_pool(name="scr", bufs=2) as scrp,
            tc.tile_pool(name="acc", bufs=1) as accp,
            tc.tile_pool(name="tgt", bufs=1) as tp,
        ):
            acc = accp.tile([P, nchunks + 2], f32)

            # ---- masked cells: one (24, 3*NC) block, columns [u | q | T] ----
            t24 = tp.tile([24, 3 * NC], f32)
            nc.scalar.dma_start(t24[:], tin_t.ap()[:])
            sg = tp.tile([24, 2 * NC], f32)
            nc.scalar.activation(sg[:], t24[:, 0:2 * NC], Act.Sigmoid)
            rc = tp.tile([24, NC], f32)
            nc.vector.reciprocal(rc[:], sg[:, NC:2 * NC])
            fm = tp.tile([24, NC], f32)
            nc.vector.scalar_tensor_tensor(
                out=fm[:], in0=rc[:], scalar=-1.0, in1=sg[:, 0:NC],
                op0=Alu.add, op1=Alu.add)
            dm = tp.tile([24, NC], f32)
            nc.vector.scalar_tensor_tensor(
                out=dm[:], in0=fm[:], scalar=0.0, in1=t24[:, 2 * NC:3 * NC],
                op0=Alu.add, op1=Alu.subtract)
            t1 = tp.tile([24, NC], f32)
            nc.vector.scalar_tensor_tensor(
                out=t1[:], in0=dm[:], scalar=0.0, in1=dm[:],
                op0=Alu.add, op1=Alu.mult,
                accum_out=acc[0:24, nchunks:nchunks + 1])
            t2 = tp.tile([24, NC], f32)
            nc.vector.scalar_tensor_tensor(
                out=t2[:], in0=fm[:], scalar=0.0, in1=fm[:],
                op0=Alu.add, op1=Alu.mult,
                accum_out=acc[0:24, nchunks + 1:nchunks + 2])

            # ---- bulk: sum of sigmoid(conf)^2, chunked over the free dim ----
            for i in range(nchunks):
                x = xp.tile([P, cw], f32)
                nc.sync.dma_start(x[:], conf_t.ap()[:, i * cw:(i + 1) * cw])
                s = sp.tile([P, cw], f32)
                nc.scalar.activation(s[:], x[:], Act.Sigmoid)
                sq = scrp.tile([P, cw], f32)
                nc.vector.scalar_tensor_tensor(
                    out=sq[:], in0=s[:], scalar=0.0, in1=s[:],
                    op0=Alu.add, op1=Alu.mult, accum_out=acc[:, i:i + 1])

            nc.sync.dma_start(oall_t.ap()[:], acc[:])

    nc.compile()
    return nc


def _get_program(ncells_pad, nchunks):
    key = (ncells_pad, nchunks)
    if key not in _PROGRAM_CACHE:
        _PROGRAM_CACHE[key] = _build_program(ncells_pad, nchunks)
    return _PROGRAM_CACHE[key]


def kernel(pred, targets):
    global LAST
    from concourse.bass_utils import run_bass_kernel_spmd

    pred = np.ascontiguousarray(np.asarray(pred, dtype=np.float32))
    targets = np.asarray(targets, dtype=np.float32)
    assert pred.shape == (B, C, H, W), pred.shape
    N = targets.shape[0]

    # ---- host: parse targets, dedupe cells (last writer wins) ----
    b = targets[:, 0].astype(np.int32)
    c = targets[:, 1].astype(np.int32)
    gix = (targets[:, 2] * W).astype(np.int32)
    giy = (targets[:, 3] * H).astype(np.int32)
    valid = (gix < W) & (giy < H) & (gix >= 0) & (giy >= 0) & (b >= 0) & (b < B)

    cell_map = {}
    for i in range(N):
        if valid[i]:
            cell_map[(int(b[i]), int(giy[i]), int(gix[i]))] = i
    n_cells = len(cell_map)
    n = 3.0 * n_cells

    per_core = [[] for _ in range(M)]
    for (bb, yy, xx), i in cell_map.items():
        per_core[bb // BPC].append((bb, yy, xx, i))

    max_cells = max((len(pc) for pc in per_core), default=0)
    ncells_pad = max(32, ((max_cells + 31) // 32) * 32)

    # ---- host: build per-core shards ----
    pr = pred.reshape(B, A, 8, H, W)
    conf_all = pr[:, :, 4, :, :]  # (B, A, H, W)

    SIG_COL = np.array([k in (0, 1, 4, 5, 6, 7) for k in range(8)] * A)  # (24,)

    NC = ncells_pad
    in_maps = []
    for m in range(M):
        shard = np.ascontiguousarray(
            conf_all[m * BPC:(m + 1) * BPC]).reshape(P, FREE)

        cells = per_core[m]
        tin = np.empty((24, 3 * NC), np.float32)
        tin[:, 0:NC] = NEG      # u pad -> sig = 0
        tin[:, NC:2 * NC] = -NEG  # q pad -> sig = 1 -> 1/sig - 1 = 0
        tin[:, 2 * NC:] = 0.0   # T pad
        if cells:
            bbs = np.array([e[0] for e in cells])
            yys = np.array([e[1] for e in cells])
            xxs = np.array([e[2] for e in cells])
            idx = np.array([e[3] for e in cells])
            vals = pred[bbs, :, yys, xxs].T  # (24, ncells)
            ncol = len(cells)
            tin[:, 0:ncol] = np.where(SIG_COL[:, None], vals, NEG)
            tin[:, NC:NC + ncol] = np.where(SIG_COL[:, None], -NEG, -vals)
            boxes = targets[idx, 2:6].T  # (4, ncells): gx, gy, gw, gh
            onehot = np.zeros((NUM_CLS, ncol), np.float32)
            ci = c[idx]
            ok = (ci >= 0) & (ci < NUM_CLS)
            onehot[ci[ok], np.nonzero(ok)[0]] = 1.0
            t0 = 2 * NC
            for a in range(A):
                tin[a * 8 + 0:a * 8 + 4, t0:t0 + ncol] = boxes
                tin[a * 8 + 4, t0:t0 + ncol] = 1.0
                tin[a * 8 + 5:a * 8 + 8, t0:t0 + ncol] = onehot
        in_maps.append({"conf": shard, "tin": tin})

    # ---- device ----
    nc = _get_program(ncells_pad, NCHUNKS)
    res = run_bass_kernel_spmd(nc, in_maps, list(range(M)), trace=TRACE)
    LAST = res

    # ---- host: combine ----
    S2 = 0.0
    r1_tot = np.zeros(24, np.float64)
    r2_tot = np.zeros(24, np.float64)
    for m in range(M):
        out = res.results[m]["oall"].astype(np.float64)
        S2 += out[:, :NCHUNKS].sum()
        r1_tot += out[0:24, NCHUNKS]
        r2_tot += out[0:24, NCHUNKS + 1]

    box_rows = [a * 8 + k for a in range(A) for k in range(4)]
    conf_rows = [a * 8 + 4 for a in range(A)]
    cls_rows = [a * 8 + k for a in range(A) for k in range(5, 8)]

    box_sum = r1_tot[box_rows].sum()
    cls_sum = r1_tot[cls_rows].sum()
    conf_corr = (r1_tot[conf_rows] - r2_tot[conf_rows]).sum()

    with np.errstate(divide="ignore", invalid="ignore"):
        loss_box = box_sum / (n * 4.0)
        loss_conf = (S2 + conf_corr) / float(B * A * HW)
        loss_cls = cls_sum / (n * NUM_CLS)
        total = 5.0 * loss_box + loss_conf + loss_cls
    return np.asarray(total, dtype=np.float32)


# revision 24
# speedup vs baseline: 1.7234x; 1.0332x over previous
"""Trainium2 Bass kernel for the YOLO-style DetectionLoss.

Full inputs in, full (scalar) output out. Internally:
  - Only the conf channels (a*8+4, i.e. 3 of 24 channels) need a full-tensor
    pass: loss_conf = mean((sigmoid(conf) - m)^2). Decompose as
       sum_all sigmoid(conf)^2  +  sum_masked [(sig-1)^2 - sig^2]
    so the bulk device work is an 8-way batch-sharded sigmoid-square-reduce
    over pred[:, 4::8] (1.23 MB/core instead of 9.8 MB/core for full pred).
  - The masked box/cls/conf-correction terms only touch the <=512 target
    cells; those 24 values/cell are gathered host-side (pure indexing) and
    evaluated on-device in one small (24, 3*ncells) block per core laid out
    as [u | q | T] along the free axis. To keep the ACT engine on a single
    function table (table switches cost ~1.3us), exp(v) is computed as
    1/sigmoid(-v) - 1 via the DVE reciprocal:
       u = v on sigmoid-cols else -100    -> sig(u) = sig(v) or 0
       q = -v on exp-cols   else +100     -> 1/sig(q) - 1 = e^v or 0
       F = (1/sig(q) - 1) + sig(u);  per-channel-row reductions:
       r1[row] = sum_cells (F - T)^2 ;  r2[row] = sum_cells F^2
  - Host combines the 8 cores' partial sums and applies the final divisions.

Perf notes (measured on trn2 via ntff profiles):
  - The stock TileContext tail (drain + EVSEM butterfly + sem clear +
    butterfly) serializes every engine at the end, and the runtime epilogue
    then resets the full 256-sem space (~50 sems/engine, ~60-115ns each)
    AFTER that rendezvous. Skipping the Tile tail entirely lets idle
    engines run their epilogue resets during the kernel body. The runtime
    epilogue re-zeroes every semaphore, so re-execution stays correct.
  - A second act-table load (set 0) is inserted at entry by the fixpoint
    pass even though only the sigmoid table (set 2) is used; it is dropped.
"""

import numpy as np

A = 3
NUM_CLS = 3
B, C, H, W = 32, 24, 160, 160
HW = H * W
M = 8            # cores
BPC = B // M     # batches per core
P = 128
CONF_ELEMS = BPC * A * HW        # 307200 per core
FREE = CONF_ELEMS // P           # 2400
NEG = -100.0                     # sigmoid(-100) == 0, sigmoid(+100) == 1 in f32

CHUNKS = (256, 768, 768, 608)   # small first chunk hides the first DMA receipt
TAIL_MODE = 2      # 0 = stock Tile tail; 1 = sem-only barrier; 2 = no tail
DROP_TABLE0 = True

TRACE = False        # test harness can flip this to get a profile
LAST = None          # BassKernelResults of the most recent run

_PROGRAM_CACHE = {}


def _make_tile_context(nc):
    import concourse.tile as tile
    from concourse.vector_clock import ScopedClock

    class _FastTailTileContext(tile.TileContext):
        def _drain_and_barrier(self, tick_clock, wait_clock):
            if TAIL_MODE == 0:
                return super()._drain_and_barrier(tick_clock, wait_clock)
            if TAIL_MODE == 1:
                drain_inst = self.nc.sync.drain()
                wait_clock.add_sem_waits(
                    drain_inst.ins, ScopedClock({None: tick_clock.global_clock})
                )
                self.nc.all_engine_barrier(sem_only=True)
                popped = self.nc._tile_sem_poison_stack.pop()
                assert popped is self._sem_poison
                self.nc.clear_and_free_semaphores(
                    list(self.sems.allocated().values())
                )
                return
            # TAIL_MODE == 2: no in-kernel tail at all. In-body semaphores
            # already order every data dependency (incl. the output DMA);
            # NEFF completion itself waits for engine streams + DMA queues,
            # and the runtime epilogue zeroes the whole semaphore space.
            popped = self.nc._tile_sem_poison_stack.pop()
            assert popped is self._sem_poison

    return _FastTailTileContext(nc)


def _make_bacc():
    from concourse import bacc, mybir

    class _Bacc(bacc.Bacc):
        def insert_act_table_loads(self):
            super().insert_act_table_loads()
            if not DROP_TABLE0:
                return
            # The entry-state fixpoint conservatively loads table set 0, but
            # every activation here is Sigmoid (set 2), which gets its own
            # load. Drop the set-0 load (1.28us on the ACT engine).
            for blk in self.main_func.blocks:
                keep = []
                for inst in blk.instructions:
                    if (
                        isinstance(inst, mybir.InstLoadActFuncSet)
                        and inst.act_func_set_id == 0
                        and not (
                            inst.sync_info
                            and (inst.sync_info.on_wait or inst.sync_info.on_update)
                        )
                    ):
                        continue
                    keep.append(inst)
                blk.instructions[:] = keep

    return _Bacc("TRN2", target_bir_lowering=False, debug=False, num_devices=M)


def _build_program(ncells_pad, chunks):
    from concourse import mybir

    f32 = mybir.dt.float32
    bf16 = mybir.dt.bfloat16
    Act = mybir.ActivationFunctionType
    Alu = mybir.AluOpType

    nc = _make_bacc()
    nchunks = len(chunks)

    NC = ncells_pad
    conf_t = nc.dram_tensor("conf", [P, FREE], bf16, kind="ExternalInput")
    # columns [0:NC]=u, [NC:2NC]=q, [2NC:3NC]=T; rows 24-127 are padding so
    # the DMA uses the fast full-128-partition descriptor path
    tin_t = nc.dram_tensor("tin", [P, 3 * NC], f32, kind="ExternalInput")
    oall_t = nc.dram_tensor("oall", [P, nchunks + 2], f32, kind="ExternalOutput")

    assert sum(chunks) == FREE

    with _make_tile_context(nc) as tc:
        with (
            tc.tile_pool(name="x", bufs=4) as xp,
            tc.tile_pool(name="s", bufs=3) as sp,
            tc.tile_pool(name="scr", bufs=2) as scrp,
            tc.tile_pool(name="acc", bufs=1) as accp,
            tc.tile_pool(name="tgt", bufs=1) as tp,
        ):
            acc = accp.tile([P, nchunks + 2], f32)

            # ---- masked cells: one (128, 3*NC) block, columns [u | q | T] ----
            t24 = tp.tile([P, 3 * NC], f32)
            nc.scalar.dma_start(t24[:], tin_t.ap()[:])
            sg = tp.tile([P, 2 * NC], f32)
            nc.scalar.activation(sg[:], t24[:, 0:2 * NC], Act.Sigmoid)
            rc = tp.tile([24, NC], f32)
            nc.vector.reciprocal(rc[:], sg[0:24, NC:2 * NC])
            fm = tp.tile([24, NC], f32)
            nc.vector.scalar_tensor_tensor(
                out=fm[:], in0=rc[:], scalar=-1.0, in1=sg[0:24, 0:NC],
                op0=Alu.add, op1=Alu.add)
            dm = tp.tile([24, NC], f32)
            nc.vector.scalar_tensor_tensor(
                out=dm[:], in0=fm[:], scalar=0.0, in1=t24[0:24, 2 * NC:3 * NC],
                op0=Alu.add, op1=Alu.subtract)
            t1 = tp.tile([24, NC], f32)
            nc.vector.scalar_tensor_tensor(
                out=t1[:], in0=dm[:], scalar=0.0, in1=dm[:],
                op0=Alu.add, op1=Alu.mult,
                accum_out=acc[0:24, nchunks:nchunks + 1])
            t2 = tp.tile([24, NC], f32)
            nc.vector.scalar_tensor_tensor(
                out=t2[:], in0=fm[:], scalar=0.0, in1=fm[:],
                op0=Alu.add, op1=Alu.mult,
                accum_out=acc[0:24, nchunks + 1:nchunks + 2])

            # ---- bulk: sum of sigmoid(conf)^2, chunked over the free dim ----
            col = 0
            for i, cw in enumerate(chunks):
                x = xp.tile([P, cw], bf16, tag="x")
                nc.sync.dma_start(x[:], conf_t.ap()[:, col:col + cw])
                s = sp.tile([P, cw], bf16, tag="s")
                nc.scalar.activation(s[:], x[:], Act.Sigmoid)
                sq = scrp.tile([P, cw], f32, tag="sq")
                nc.vector.scalar_tensor_tensor(
                    out=sq[:], in0=s[:], scalar=0.0, in1=s[:],
                    op0=Alu.add, op1=Alu.mult, accum_out=acc[:, i:i + 1])
                col += cw

            nc.sync.dma_start(oall_t.ap()[:], acc[:])

    nc.compile()
    return nc


def _get_program(ncells_pad, chunks):
    key = (ncells_pad, chunks)
    if key not in _PROGRAM_CACHE:
        _PROGRAM_CACHE[key] = _build_program(ncells_pad, chunks)
    return _PROGRAM_CACHE[key]


def kernel(pred, targets):
    global LAST
    from concourse.bass_utils import run_bass_kernel_spmd

    pred = np.ascontiguousarray(np.asarray(pred, dtype=np.float32))
    targets = np.asarray(targets, dtype=np.float32)
    assert pred.shape == (B, C, H, W), pred.shape
    N = targets.shape[0]

    # ---- host: parse targets, dedupe cells (last writer wins) ----
    b = targets[:, 0].astype(np.int32)
    c = targets[:, 1].astype(np.int32)
    gix = (targets[:, 2] * W).astype(np.int32)
    giy = (targets[:, 3] * H).astype(np.int32)
    valid = (gix < W) & (giy < H) & (gix >= 0) & (giy >= 0) & (b >= 0) & (b < B)

    cell_map = {}
    for i in range(N):
        if valid[i]:
            cell_map[(int(b[i]), int(giy[i]), int(gix[i]))] = i
    n_cells = len(cell_map)
    n = 3.0 * n_cells

    per_core = [[] for _ in range(M)]
    for (bb, yy, xx), i in cell_map.items():
        per_core[bb // BPC].append((bb, yy, xx, i))

    max_cells = max((len(pc) for pc in per_core), default=0)
    ncells_pad = max(32, ((max_cells + 31) // 32) * 32)

    # ---- host: build per-core shards ----
    pr = pred.reshape(B, A, 8, H, W)
    conf_all = pr[:, :, 4, :, :]  # (B, A, H, W)

    SIG_COL = np.array([k in (0, 1, 4, 5, 6, 7) for k in range(8)] * A)  # (24,)

    import ml_dtypes
    NC = ncells_pad
    in_maps = []
    for m in range(M):
        shard = np.ascontiguousarray(
            conf_all[m * BPC:(m + 1) * BPC]).reshape(P, FREE).astype(
                ml_dtypes.bfloat16)

        cells = per_core[m]
        tin = np.empty((P, 3 * NC), np.float32)
        tin[:, 0:NC] = NEG      # u pad -> sig = 0
        tin[:, NC:2 * NC] = -NEG  # q pad -> sig = 1 -> 1/sig - 1 = 0
        tin[:, 2 * NC:] = 0.0   # T pad
        if cells:
            bbs = np.array([e[0] for e in cells])
            yys = np.array([e[1] for e in cells])
            xxs = np.array([e[2] for e in cells])
            idx = np.array([e[3] for e in cells])
            vals = pred[bbs, :, yys, xxs].T  # (24, ncells)
            ncol = len(cells)
            tin[0:24, 0:ncol] = np.where(SIG_COL[:, None], vals, NEG)
            tin[0:24, NC:NC + ncol] = np.where(SIG_COL[:, None], -NEG, -vals)
            boxes = targets[idx, 2:6].T  # (4, ncells): gx, gy, gw, gh
            onehot = np.zeros((NUM_CLS, ncol), np.float32)
            ci = c[idx]
            ok = (ci >= 0) & (ci < NUM_CLS)
            onehot[ci[ok], np.nonzero(ok)[0]] = 1.0
            t0 = 2 * NC
            for a in range(A):
                tin[a * 8 + 0:a * 8 + 4, t0:t0 + ncol] = boxes
                tin[a * 8 + 4, t0:t0 + ncol] = 1.0
                tin[a * 8 + 5:a * 8 + 8, t0:t0 + ncol] = onehot
        in_maps.append({"conf": shard, "tin": tin})

    # ---- device ----
    nchunks = len(CHUNKS)
    nc = _get_program(ncells_pad, CHUNKS)
    res = run_bass_kernel_spmd(nc, in_maps, list(range(M)), trace=TRACE)
    LAST = res

    # ---- host: combine ----
    S2 = 0.0
    r1_tot = np.zeros(24, np.float64)
    r2_tot = np.zeros(24, np.float64)
    for m in range(M):
        out = res.results[m]["oall"].astype(np.float64)
        S2 += out[:, :nchunks].sum()
        r1_tot += out[0:24, nchunks]
        r2_tot += out[0:24, nchunks + 1]

    box_rows = [a * 8 + k for a in range(A) for k in range(4)]
    conf_rows = [a * 8 + 4 for a in range(A)]
    cls_rows = [a * 8 + k for a in range(A) for k in range(5, 8)]

    box_sum = r1_tot[box_rows].sum()
    cls_sum = r1_tot[cls_rows].sum()
    conf_corr = (r1_tot[conf_rows] - r2_tot[conf_rows]).sum()

    with np.errstate(divide="ignore", invalid="ignore"):
        loss_box = box_sum / (n * 4.0)
        loss_conf = (S2 + conf_corr) / float(B * A * HW)
        loss_cls = cls_sum / (n * NUM_CLS)
        total = 5.0 * loss_box + loss_conf + loss_cls
    return np.asarray(total, dtype=np.float32)


# revision 27
# speedup vs baseline: 1.7826x; 1.0344x over previous
"""Trainium2 Bass kernel for the YOLO-style DetectionLoss.

Full inputs in, full (scalar) output out. Internally:
  - Only the conf channels (a*8+4, i.e. 3 of 24 channels) need a full-tensor
    pass: loss_conf = mean((sigmoid(conf) - m)^2). Decompose as
       sum_all sigmoid(conf)^2  +  sum_masked [(sig-1)^2 - sig^2]
    so the bulk device work is an 8-way batch-sharded sigmoid-square-reduce
    over pred[:, 4::8] (1.23 MB/core instead of 9.8 MB/core for full pred).
  - The masked box/cls/conf-correction terms only touch the <=512 target
    cells; those 24 values/cell are gathered host-side (pure indexing) and
    evaluated on-device in one small (24, 3*ncells) block per core laid out
    as [u | q | T] along the free axis. To keep the ACT engine on a single
    function table (table switches cost ~1.3us), exp(v) is computed as
    1/sigmoid(-v) - 1 via the DVE reciprocal:
       u = v on sigmoid-cols else -100    -> sig(u) = sig(v) or 0
       q = -v on exp-cols   else +100     -> 1/sig(q) - 1 = e^v or 0
       F = (1/sig(q) - 1) + sig(u);  per-channel-row reductions:
       r1[row] = sum_cells (F - T)^2 ;  r2[row] = sum_cells F^2
  - Host combines the 8 cores' partial sums and applies the final divisions.

Perf notes (measured on trn2 via ntff profiles):
  - The stock TileContext tail (drain + EVSEM butterfly + sem clear +
    butterfly) serializes every engine at the end, and the runtime epilogue
    then resets the full 256-sem space (~50 sems/engine, ~60-115ns each)
    AFTER that rendezvous. Skipping the Tile tail entirely lets idle
    engines run their epilogue resets during the kernel body. The runtime
    epilogue re-zeroes every semaphore, so re-execution stays correct.
  - A second act-table load (set 0) is inserted at entry by the fixpoint
    pass even though only the sigmoid table (set 2) is used; it is dropped.
"""

import numpy as np

A = 3
NUM_CLS = 3
B, C, H, W = 32, 24, 160, 160
HW = H * W
M = 8            # cores
BPC = B // M     # batches per core
P = 128
CONF_ELEMS = BPC * A * HW        # 307200 per core
FREE = CONF_ELEMS // P           # 2400
NEG = -100.0                     # sigmoid(-100) == 0, sigmoid(+100) == 1 in f32

CHUNKS = (256, 768, 768, 608)   # small first chunk hides the first DMA receipt
TAIL_MODE = 2      # 0 = stock Tile tail; 1 = sem-only barrier; 2 = no tail
DROP_TABLE0 = True

TRACE = False        # test harness can flip this to get a profile
LAST = None          # BassKernelResults of the most recent run

_PROGRAM_CACHE = {}


def _make_tile_context(nc):
    import concourse.tile as tile
    from concourse.vector_clock import ScopedClock

    class _FastTailTileContext(tile.TileContext):
        def _drain_and_barrier(self, tick_clock, wait_clock):
            if TAIL_MODE == 0:
                return super()._drain_and_barrier(tick_clock, wait_clock)
            if TAIL_MODE == 1:
                drain_inst = self.nc.sync.drain()
                wait_clock.add_sem_waits(
                    drain_inst.ins, ScopedClock({None: tick_clock.global_clock})
                )
                self.nc.all_engine_barrier(sem_only=True)
                popped = self.nc._tile_sem_poison_stack.pop()
                assert popped is self._sem_poison
                self.nc.clear_and_free_semaphores(
                    list(self.sems.allocated().values())
                )
                return
            # TAIL_MODE == 2: no in-kernel tail at all. In-body semaphores
            # already order every data dependency (incl. the output DMA);
            # NEFF completion itself waits for engine streams + DMA queues,
            # and the runtime epilogue zeroes the whole semaphore space.
            popped = self.nc._tile_sem_poison_stack.pop()
            assert popped is self._sem_poison

    return _FastTailTileContext(nc)


def _make_bacc():
    from concourse import bacc, mybir

    class _Bacc(bacc.Bacc):
        def __init__(self, *a, **kw):
            # Skip the const-memset all-engine barrier Bass.__init__ emits
            # (~1us on the critical path). The only consumer of those const
            # tiles here is the activation bias, which we replace with a
            # tile-tracked zero tile inside the TileContext.
            self._skip_init_barrier = True
            super().__init__(*a, **kw)
            self._skip_init_barrier = False

        def all_engine_barrier(self, *, sem_only: bool = False):
            if getattr(self, "_skip_init_barrier", False):
                return
            super().all_engine_barrier(sem_only=sem_only)

        def insert_act_table_loads(self):
            super().insert_act_table_loads()
            if not DROP_TABLE0:
                return
            # The entry-state fixpoint conservatively loads table set 0, but
            # every activation here is Sigmoid (set 2), which gets its own
            # load. Drop the set-0 load (1.28us on the ACT engine).
            for blk in self.main_func.blocks:
                keep = []
                for inst in blk.instructions:
                    if (
                        isinstance(inst, mybir.InstLoadActFuncSet)
                        and inst.act_func_set_id == 0
                        and not (
                            inst.sync_info
                            and (inst.sync_info.on_wait or inst.sync_info.on_update)
                        )
                    ):
                        continue
                    keep.append(inst)
                blk.instructions[:] = keep

    return _Bacc("TRN2", target_bir_lowering=False, debug=False, num_devices=M)


def _build_program(ncells_pad, chunks):
    from concourse import mybir

    f32 = mybir.dt.float32
    bf16 = mybir.dt.bfloat16
    Act = mybir.ActivationFunctionType
    Alu = mybir.AluOpType

    nc = _make_bacc()
    nchunks = len(chunks)

    NC = ncells_pad
    conf_t = nc.dram_tensor("conf", [P, FREE], bf16, kind="ExternalInput")
    # columns [0:NC]=u, [NC:2NC]=q, [2NC:3NC]=T; rows 24-127 are padding so
    # the DMA uses the fast full-128-partition descriptor path
    tin_t = nc.dram_tensor("tin", [P, 3 * NC], f32, kind="ExternalInput")
    oall_t = nc.dram_tensor("oall", [P, nchunks + 2], f32, kind="ExternalOutput")

    assert sum(chunks) == FREE

    with _make_tile_context(nc) as tc:
        with (
            tc.tile_pool(name="x", bufs=4) as xp,
            tc.tile_pool(name="s", bufs=3) as sp,
            tc.tile_pool(name="scr", bufs=2) as scrp,
            tc.tile_pool(name="acc", bufs=1) as accp,
            tc.tile_pool(name="tgt", bufs=1) as tp,
        ):
            acc = accp.tile([P, nchunks + 2], f32)

            # zero bias tile for every activation (replaces the global const
            # tile whose init barrier we skip; Tile orders the memset first)
            zb = accp.tile([P, 1], f32)
            nc.gpsimd.memset(zb[:], 0.0)

            # ---- masked cells: one (128, 3*NC) block, columns [u | q | T] ----
            t24 = tp.tile([P, 3 * NC], f32)
            nc.scalar.dma_start(t24[:], tin_t.ap()[:])
            sg = tp.tile([P, 2 * NC], f32)
            nc.scalar.activation(sg[:], t24[:, 0:2 * NC], Act.Sigmoid, bias=zb[:])
            rc = tp.tile([24, NC], f32)
            nc.vector.reciprocal(rc[:], sg[0:24, NC:2 * NC])
            fm = tp.tile([24, NC], f32)
            nc.vector.scalar_tensor_tensor(
                out=fm[:], in0=rc[:], scalar=-1.0, in1=sg[0:24, 0:NC],
                op0=Alu.add, op1=Alu.add)
            dm = tp.tile([24, NC], f32)
            nc.vector.scalar_tensor_tensor(
                out=dm[:], in0=fm[:], scalar=0.0, in1=t24[0:24, 2 * NC:3 * NC],
                op0=Alu.add, op1=Alu.subtract)
            t1 = tp.tile([24, NC], f32)
            nc.vector.scalar_tensor_tensor(
                out=t1[:], in0=dm[:], scalar=0.0, in1=dm[:],
                op0=Alu.add, op1=Alu.mult,
                accum_out=acc[0:24, nchunks:nchunks + 1])
            t2 = tp.tile([24, NC], f32)
            nc.vector.scalar_tensor_tensor(
                out=t2[:], in0=fm[:], scalar=0.0, in1=fm[:],
                op0=Alu.add, op1=Alu.mult,
                accum_out=acc[0:24, nchunks + 1:nchunks + 2])

            # ---- bulk: sum of sigmoid(conf)^2, chunked over the free dim ----
            col = 0
            for i, cw in enumerate(chunks):
                x = xp.tile([P, cw], bf16, tag="x")
                nc.sync.dma_start(x[:], conf_t.ap()[:, col:col + cw])
                s = sp.tile([P, cw], bf16, tag="s")
                nc.scalar.activation(s[:], x[:], Act.Sigmoid, bias=zb[:])
                sq = scrp.tile([P, cw], bf16, tag="sq")
                nc.vector.scalar_tensor_tensor(
                    out=sq[:], in0=s[:], scalar=0.0, in1=s[:],
                    op0=Alu.add, op1=Alu.mult, accum_out=acc[:, i:i + 1])
                col += cw

            nc.sync.dma_start(oall_t.ap()[:], acc[:])

    nc.compile()
    return nc


def _get_program(ncells_pad, chunks):
    key = (ncells_pad, chunks)
    if key not in _PROGRAM_CACHE:
        _PROGRAM_CACHE[key] = _build_program(ncells_pad, chunks)
    return _PROGRAM_CACHE[key]


def kernel(pred, targets):
    global LAST
    from concourse.bass_utils import run_bass_kernel_spmd

    pred = np.ascontiguousarray(np.asarray(pred, dtype=np.float32))
    targets = np.asarray(targets, dtype=np.float32)
    assert pred.shape == (B, C, H, W), pred.shape
    N = targets.shape[0]

    # ---- host: parse targets, dedupe cells (last writer wins) ----
    b = targets[:, 0].astype(np.int32)
    c = targets[:, 1].astype(np.int32)
    gix = (targets[:, 2] * W).astype(np.int32)
    giy = (targets[:, 3] * H).astype(np.int32)
    valid = (gix < W) & (giy < H) & (gix >= 0) & (giy >= 0) & (b >= 0) & (b < B)

    cell_map = {}
    for i in range(N):
        if valid[i]:
            cell_map[(int(b[i]), int(giy[i]), int(gix[i]))] = i
    n_cells = len(cell_map)
    n = 3.0 * n_cells

    per_core = [[] for _ in range(M)]
    for (bb, yy, xx), i in cell_map.items():
        per_core[bb // BPC].append((bb, yy, xx, i))

    max_cells = max((len(pc) for pc in per_core), default=0)
    ncells_pad = max(32, ((max_cells + 31) // 32) * 32)

    # ---- host: build per-core shards ----
    pr = pred.reshape(B, A, 8, H, W)
    conf_all = pr[:, :, 4, :, :]  # (B, A, H, W)

    SIG_COL = np.array([k in (0, 1, 4, 5, 6, 7) for k in range(8)] * A)  # (24,)

    import ml_dtypes
    NC = ncells_pad
    in_maps = []
    for m in range(M):
        shard = np.ascontiguousarray(
            conf_all[m * BPC:(m + 1) * BPC]).reshape(P, FREE).astype(
                ml_dtypes.bfloat16)

        cells = per_core[m]
        tin = np.empty((P, 3 * NC), np.float32)
        tin[:, 0:NC] = NEG      # u pad -> sig = 0
        tin[:, NC:2 * NC] = -NEG  # q pad -> sig = 1 -> 1/sig - 1 = 0
        tin[:, 2 * NC:] = 0.0   # T pad
        if cells:
            bbs = np.array([e[0] for e in cells])
            yys = np.array([e[1] for e in cells])
            xxs = np.array([e[2] for e in cells])
            idx = np.array([e[3] for e in cells])
            vals = pred[bbs, :, yys, xxs].T  # (24, ncells)
            ncol = len(cells)
            tin[0:24, 0:ncol] = np.where(SIG_COL[:, None], vals, NEG)
            tin[0:24, NC:NC + ncol] = np.where(SIG_COL[:, None], -NEG, -vals)
            boxes = targets[idx, 2:6].T  # (4, ncells): gx, gy, gw, gh
            onehot = np.zeros((NUM_CLS, ncol), np.float32)
            ci = c[idx]
            ok = (ci >= 0) & (ci < NUM_CLS)
            onehot[ci[ok], np.nonzero(ok)[0]] = 1.0
            t0 = 2 * NC
            for a in range(A):
                tin[a * 8 + 0:a * 8 + 4, t0:t0 + ncol] = boxes
                tin[a * 8 + 4, t0:t0 + ncol] = 1.0
                tin[a * 8 + 5:a * 8 + 8, t0:t0 + ncol] = onehot
        in_maps.append({"conf": shard, "tin": tin})

    # ---- device ----
    nchunks = len(CHUNKS)
    nc = _get_program(ncells_pad, CHUNKS)
    res = run_bass_kernel_spmd(nc, in_maps, list(range(M)), trace=TRACE)
    LAST = res

    # ---- host: combine ----
    S2 = 0.0
    r1_tot = np.zeros(24, np.float64)
    r2_tot = np.zeros(24, np.float64)
    for m in range(M):
        out = res.results[m]["oall"].astype(np.float64)
        S2 += out[:, :nchunks].sum()
        r1_tot += out[0:24, nchunks]
        r2_tot += out[0:24, nchunks + 1]

    box_rows = [a * 8 + k for a in range(A) for k in range(4)]
    conf_rows = [a * 8 + 4 for a in range(A)]
    cls_rows = [a * 8 + k for a in range(A) for k in range(5, 8)]

    box_sum = r1_tot[box_rows].sum()
    cls_sum = r1_tot[cls_rows].sum()
    conf_corr = (r1_tot[conf_rows] - r2_tot[conf_rows]).sum()

    with np.errstate(divide="ignore", invalid="ignore"):
        loss_box = box_sum / (n * 4.0)
        loss_conf = (S2 + conf_corr) / float(B * A * HW)
        loss_cls = cls_sum / (n * NUM_CLS)
        total = 5.0 * loss_box + loss_conf + loss_cls
    return np.asarray(total, dtype=np.float32)
